# revision 13
# baseline (speedup 1.0000x reference)
"""Trainium2 Bass kernel for the LocalConnectivity diamond-ring stencil.

out[b, x, y] = sum_{1<=|dx|+|dy|<=5} w[|dx|+|dy|-1] * in[b, (x+dx)%512, (y+dy)%512]

Strategy
--------
Data-parallel over batch: 64 samples -> 8 cores x 8 samples. Per sample the
512x512 grid is processed in 5 row-tiles (~103 output rows each). The whole
60-tap stencil runs on the TensorEngine as 11 PSUM-accumulating matmuls, one
per horizontal shift dy in [-5, 5]:

  psum[p, f] += sum_c  WB_dy[c, p] * X[c, f + dy_idx]

where X is the input tile with 5 halo rows on each side (contraction dim =
nrows+10 partitions) and 5 circular halo columns on each side (horizontal
shifts become free-dim AP offsets), and WB_dy is the banded Toeplitz matrix
holding the vertical taps of kernel column dy: WB_dy[c, p] = K(c-p-5, dy).

v2: all-bf16 matmuls. f32r weights cannot use FWL or the background weight
buffer, which exposed ~157ns of LDWEIGHTS serialization per matmul (~68us
of the 200us baseline). bf16 weights padded to the full 128 columns satisfy
the FWL trigger (NumWeights==128, non-fp32) and LDWEIGHTS prefetches into
the background weight buffer during the previous matmul. Mixed bf16 x f32r
is rejected by the NEFF compiler, so the moving data is converted f32->bf16
on ScalarE (one extra 113x522 copy per tile, ~0.5us, off the critical
engine). PSUM rows nrows..127 accumulate garbage from the zero-padded band
columns and are never evicted. A dozen warmup matmuls on the weight tile
ramp the PE p-state while the first input tiles are still in flight on DMA.

Bulk HBM traffic is issued from GpSimd (software DGE - the only DGE that
fans transfers out across all 16 SDMA engines; the sync/scalar HW-DGE queues
each serialize on one SDMA engine at ~15-18 GB/s). Transfers stay per-tile
so consecutive DMAs round-robin onto different SDMA queues. Circular column
halos are filled by on-chip ScalarE copies; PSUM eviction runs on VectorE.
"""

import numpy as np
import ml_dtypes

import concourse.bass as bass
import concourse.bacc as bacc
import concourse.mybir as mybir
from concourse import tile
from concourse.bass_utils import run_bass_kernel_spmd

B, H, W = 64, 512, 512
NCORES = 8
BPC = B // NCORES  # samples per core
MAXD = 5
HALO = MAXD
DYS = 2 * MAXD + 1  # 11 horizontal shifts
TR = 103  # rows per tile (last tile: 100)
ROW_TILES = [(0, 103), (103, 103), (206, 103), (309, 103), (412, 100)]
XW = W + 2 * HALO  # 522
WARMUP_MM = 12


def _build_band_weights(dw: np.ndarray) -> np.ndarray:
    """[128, 11*128] bf16: WB[c, j*128 + p] = K(c-p-5, j-5)."""
    wb = np.zeros((128, DYS, 128), dtype=np.float32)
    p = np.arange(128)
    for j in range(DYS):
        dy = j - MAXD
        for dx in range(-MAXD, MAXD + 1):
            d = abs(dx) + abs(dy)
            if 1 <= d <= MAXD:
                c = p + dx + HALO
                valid = (c >= 0) & (c < 128)
                wb[c[valid], j, p[valid]] = dw[d - 1]
    return np.ascontiguousarray(
        wb.reshape(128, DYS * 128).astype(ml_dtypes.bfloat16)
    )


_CACHED_NC = None


def _build_program():
    f32 = mybir.dt.float32
    bf16 = mybir.dt.bfloat16

    nc = bacc.Bacc(None, target_bir_lowering=False)
    x = nc.dram_tensor("x", [BPC, H, W], f32, kind="ExternalInput")
    wb = nc.dram_tensor("wb", [128, DYS * 128], bf16, kind="ExternalInput")
    y = nc.dram_tensor("y", [BPC, H, W], f32, kind="ExternalOutput")

    with tile.TileContext(nc) as tc:
        with (
            tc.tile_pool(name="wpool", bufs=1) as wpool,
            tc.tile_pool(name="xmpool", bufs=5) as xmpool,
            tc.tile_pool(name="xbpool", bufs=10) as xbpool,
            tc.tile_pool(name="xepool", bufs=6) as xepool,
            tc.tile_pool(name="opool", bufs=4) as opool,
            tc.tile_pool(name="pspool", bufs=7, space=bass.MemorySpace.PSUM) as pspool,
        ):
            wtile = wpool.tile([128, DYS * 128], bf16)
            nc.gpsimd.dma_start(wtile[:], wb[:])

            # PE p-state warmup while the first input tiles are on DMA.
            wpt = pspool.tile([128, W], f32, tag="warm", bufs=1)
            for _ in range(WARMUP_MM):
                nc.tensor.matmul(
                    wpt[:],
                    wtile[0:128, 0:128],
                    wtile[0:128, 0:W],
                    start=True,
                    stop=True,
                )

            def issue_inputs(b):
                """Input DMAs for sample b. 2-sample lookahead keeps these
                ahead of the output DMAs on the in-order GpSimd DGE, so the
                next samples' transfers overlap this sample's compute."""
                xt0 = xepool.tile([128, XW], f32, tag="xt0", name="xt0")
                #   rows 507..511 then 0..107
                nc.sync.dma_start(
                    xt0[0:HALO, HALO : HALO + W], x[b, H - HALO : H, :]
                )
                nc.gpsimd.dma_start(
                    xt0[HALO : HALO + 108, HALO : HALO + W], x[b, 0:108, :]
                )
                # interior tiles t=1..3 in ONE 3D-AP DMA (each extra DMA
                # instruction burns one of the 8 DMASW semaphore lanes; lane
                # reuse stalls descriptor-gen until the DMA 8-back completes)
                xtm = xmpool.tile([128, 3, XW], f32, name="xtm")
                src3 = bass.AP(
                    x,
                    b * H * W + (TR - HALO) * W,
                    [[W, 113], [TR * W, 3], [1, W]],
                )
                nc.gpsimd.dma_start(xtm[0:113, 0:3, HALO : HALO + W], src3)
                xt4 = xepool.tile([128, XW], f32, tag="xt4", name="xt4")
                #   rows 407..511 then 0..4
                nc.gpsimd.dma_start(
                    xt4[0:105, HALO : HALO + W], x[b, 4 * TR - HALO : H, :]
                )
                nc.sync.dma_start(xt4[105:110, HALO : HALO + W], x[b, 0:HALO, :])
                return xt0, xtm, xt4

            pending = [issue_inputs(0), issue_inputs(1)]

            for b in range(BPC):
                xt0, xtm, xt4 = pending[b]
                if b + 2 < BPC:
                    pending.append(issue_inputs(b + 2))

                # ---- per tile: halo fill + f32->bf16 convert (ScalarE,
                # issued per-tile so tile t's matmuls depend only on tile t's
                # own DMA), 11 accumulating matmuls, eviction, output DMA ----
                otb = opool.tile([128, 5, W], f32)
                for t, (r0, nrows) in enumerate(ROW_TILES):
                    ctr = nrows + 2 * HALO

                    def s(rs, cs, _t=t, _xt0=xt0, _xt4=xt4, _xtm=xtm):
                        if _t == 0:
                            return _xt0[rs, cs]
                        if _t == 4:
                            return _xt4[rs, cs]
                        return _xtm[rs, _t - 1, cs]

                    nc.scalar.copy(
                        s(slice(0, ctr), slice(0, HALO)),
                        s(slice(0, ctr), slice(W, W + HALO)),
                    )
                    nc.scalar.copy(
                        s(slice(0, ctr), slice(HALO + W, XW)),
                        s(slice(0, ctr), slice(HALO, 2 * HALO)),
                    )
                    xb = xbpool.tile([128, XW], bf16)
                    nc.scalar.copy(xb[0:ctr, :], s(slice(0, ctr), slice(0, XW)))

                    pt = pspool.tile([128, W], f32)
                    for j in range(DYS):
                        nc.tensor.matmul(
                            pt[:],
                            wtile[0:ctr, j * 128 : (j + 1) * 128],
                            xb[0:ctr, j : j + W],
                            start=(j == 0),
                            stop=(j == DYS - 1),
                        )
                    nc.vector.tensor_copy(otb[0:nrows, t, :], pt[0:nrows, :])

                # merged output DMAs: tiles 0..3 (uniform 103 rows) in one
                # 3D-AP DMA, tile 4 separate — 2 DMASW lanes per sample
                dst4 = bass.AP(
                    y, b * H * W, [[W, TR], [TR * W, 4], [1, W]]
                )
                nc.gpsimd.dma_start(dst4, otb[0:TR, 0:4, :])
                nc.gpsimd.dma_start(
                    y[b, 4 * TR : H, :], otb[0:100, 4, :]
                )
    nc.compile()
    return nc


def _get_program():
    global _CACHED_NC
    if _CACHED_NC is None:
        _CACHED_NC = _build_program()
    return _CACHED_NC


def _run(grid_spikes, distance_weights, trace=False):
    grid_spikes = np.ascontiguousarray(np.asarray(grid_spikes, dtype=np.float32))
    distance_weights = np.asarray(distance_weights, dtype=np.float32)
    assert grid_spikes.shape == (B, H, W), grid_spikes.shape
    wb_np = _build_band_weights(distance_weights)

    nc = _get_program()
    in_maps = [
        {
            "x": np.ascontiguousarray(grid_spikes[i * BPC : (i + 1) * BPC]),
            "wb": wb_np,
        }
        for i in range(NCORES)
    ]
    res = run_bass_kernel_spmd(nc, in_maps, list(range(NCORES)), trace=trace)
    out = np.concatenate([res.results[i]["y"] for i in range(NCORES)], axis=0)
    return out.astype(np.float32, copy=False), res


def kernel(grid_spikes, distance_weights):
    out, _ = _run(grid_spikes, distance_weights, trace=False)
    return out


def kernel_traced(grid_spikes, distance_weights):
    out, res = _run(grid_spikes, distance_weights, trace=True)
    return out, res


# revision 14
# speedup vs baseline: 1.7931x; 1.7931x over previous
"""Trainium2 Bass kernel for the LocalConnectivity diamond-ring stencil.

out[b, x, y] = sum_{1<=|dx|+|dy|<=5} w[|dx|+|dy|-1] * in[b, (x+dx)%512, (y+dy)%512]

Strategy
--------
Data-parallel over batch: 64 samples -> 8 cores x 8 samples. Per sample the
512x512 grid is processed in 5 row-tiles (~103 output rows each). The whole
60-tap stencil runs on the TensorEngine as 11 PSUM-accumulating matmuls, one
per horizontal shift dy in [-5, 5]:

  psum[p, f] += sum_c  WB_dy[c, p] * X[c, f + dy_idx]

where X is the input tile with 5 halo rows on each side (contraction dim =
nrows+10 partitions) and 5 circular halo columns on each side (horizontal
shifts become free-dim AP offsets), and WB_dy is the banded Toeplitz matrix
holding the vertical taps of kernel column dy: WB_dy[c, p] = K(c-p-5, dy).

v2: all-bf16 matmuls. f32r weights cannot use FWL or the background weight
buffer, which exposed ~157ns of LDWEIGHTS serialization per matmul (~68us
of the 200us baseline). bf16 weights padded to the full 128 columns satisfy
the FWL trigger (NumWeights==128, non-fp32) and LDWEIGHTS prefetches into
the background weight buffer during the previous matmul. Mixed bf16 x f32r
is rejected by the NEFF compiler, so the moving data is converted f32->bf16
on ScalarE (one extra 113x522 copy per tile, ~0.5us, off the critical
engine). PSUM rows nrows..127 accumulate garbage from the zero-padded band
columns and are never evicted. A dozen warmup matmuls on the weight tile
ramp the PE p-state while the first input tiles are still in flight on DMA.

Bulk HBM traffic is issued from GpSimd (software DGE - the only DGE that
fans transfers out across all 16 SDMA engines; the sync/scalar HW-DGE queues
each serialize on one SDMA engine at ~15-18 GB/s). Transfers stay per-tile
so consecutive DMAs round-robin onto different SDMA queues. Circular column
halos are filled by on-chip ScalarE copies; PSUM eviction runs on VectorE.
"""

import numpy as np
import ml_dtypes

import concourse.bass as bass
import concourse.bacc as bacc
import concourse.mybir as mybir
from concourse import tile
from concourse.bass_utils import run_bass_kernel_spmd

B, H, W = 64, 512, 512
NCORES = 8
BPC = B // NCORES  # samples per core
MAXD = 5
HALO = MAXD
DYS = 2 * MAXD + 1  # 11 horizontal shifts
TR = 103  # rows per tile (last tile: 100)
ROW_TILES = [(0, 103), (103, 103), (206, 103), (309, 103), (412, 100)]
XW = W + 2 * HALO  # 522
WARMUP_MM = 12


def _build_band_weights(dw: np.ndarray) -> np.ndarray:
    """[128, 11*128] bf16: WB[c, j*128 + p] = K(c-p-5, j-5)."""
    wb = np.zeros((128, DYS, 128), dtype=np.float32)
    p = np.arange(128)
    for j in range(DYS):
        dy = j - MAXD
        for dx in range(-MAXD, MAXD + 1):
            d = abs(dx) + abs(dy)
            if 1 <= d <= MAXD:
                c = p + dx + HALO
                valid = (c >= 0) & (c < 128)
                wb[c[valid], j, p[valid]] = dw[d - 1]
    return np.ascontiguousarray(
        wb.reshape(128, DYS * 128).astype(ml_dtypes.bfloat16)
    )


_CACHED_NC = None


def _build_program():
    f32 = mybir.dt.float32
    bf16 = mybir.dt.bfloat16

    nc = bacc.Bacc(None, target_bir_lowering=False)
    x = nc.dram_tensor("x", [BPC, H, W], f32, kind="ExternalInput")
    wb = nc.dram_tensor("wb", [128, DYS * 128], bf16, kind="ExternalInput")
    y = nc.dram_tensor("y", [BPC, H, W], f32, kind="ExternalOutput")

    with tile.TileContext(nc) as tc:
        with (
            tc.tile_pool(name="wpool", bufs=1) as wpool,
            tc.tile_pool(name="xmpool", bufs=5) as xmpool,
            tc.tile_pool(name="xbpool", bufs=10) as xbpool,
            tc.tile_pool(name="xepool", bufs=6) as xepool,
            tc.tile_pool(name="opool", bufs=4) as opool,
            tc.tile_pool(name="pspool", bufs=7, space=bass.MemorySpace.PSUM) as pspool,
        ):
            wtile = wpool.tile([128, DYS * 128], bf16)
            nc.gpsimd.dma_start(wtile[:], wb[:])

            # PE p-state warmup while the first input tiles are on DMA.
            wpt = pspool.tile([128, W], f32, tag="warm", bufs=1)
            for _ in range(WARMUP_MM):
                nc.tensor.matmul(
                    wpt[:],
                    wtile[0:128, 0:128],
                    wtile[0:128, 0:W],
                    start=True,
                    stop=True,
                )

            def issue_inputs(b):
                """Input DMAs for sample b. 2-sample lookahead keeps these
                ahead of the output DMAs on the in-order GpSimd DGE, so the
                next samples' transfers overlap this sample's compute."""
                xt0 = xepool.tile([128, XW], f32, tag="xt0", name="xt0")
                #   rows 507..511 then 0..107
                nc.sync.dma_start(
                    xt0[0:HALO, HALO : HALO + W], x[b, H - HALO : H, :]
                )
                nc.gpsimd.dma_start(
                    xt0[HALO : HALO + 108, HALO : HALO + W], x[b, 0:108, :]
                )
                # interior tiles t=1..3: per-tile DMAs (merging them onto
                # one logical queue was measured 1.8x slower end-to-end)
                xtm = xmpool.tile([128, 3, XW], f32, name="xtm")
                for tt in range(3):
                    r0 = TR * (tt + 1)
                    nc.gpsimd.dma_start(
                        xtm[0:113, tt, HALO : HALO + W],
                        x[b, r0 - HALO : r0 + 108, :],
                    )
                xt4 = xepool.tile([128, XW], f32, tag="xt4", name="xt4")
                #   rows 407..511 then 0..4
                nc.gpsimd.dma_start(
                    xt4[0:105, HALO : HALO + W], x[b, 4 * TR - HALO : H, :]
                )
                nc.sync.dma_start(xt4[105:110, HALO : HALO + W], x[b, 0:HALO, :])
                return xt0, xtm, xt4

            pending = [issue_inputs(0), issue_inputs(1)]
            done_otb = {}

            def flush_outputs(bb):
                ob = done_otb.pop(bb)
                for t, (r0, nrows) in enumerate(ROW_TILES):
                    nc.gpsimd.dma_start(
                        y[bb, r0 : r0 + nrows, :], ob[0:nrows, t, :]
                    )

            for b in range(BPC):
                xt0, xtm, xt4 = pending[b]
                if b + 2 < BPC:
                    pending.append(issue_inputs(b + 2))

                # ---- per tile: halo fill + f32->bf16 convert (ScalarE,
                # issued per-tile so tile t's matmuls depend only on tile t's
                # own DMA), 11 accumulating matmuls, eviction, output DMA ----
                otb = opool.tile([128, 5, W], f32)
                for t, (r0, nrows) in enumerate(ROW_TILES):
                    ctr = nrows + 2 * HALO

                    def s(rs, cs, _t=t, _xt0=xt0, _xt4=xt4, _xtm=xtm):
                        if _t == 0:
                            return _xt0[rs, cs]
                        if _t == 4:
                            return _xt4[rs, cs]
                        return _xtm[rs, _t - 1, cs]

                    nc.scalar.copy(
                        s(slice(0, ctr), slice(0, HALO)),
                        s(slice(0, ctr), slice(W, W + HALO)),
                    )
                    nc.scalar.copy(
                        s(slice(0, ctr), slice(HALO + W, XW)),
                        s(slice(0, ctr), slice(HALO, 2 * HALO)),
                    )
                    xb = xbpool.tile([128, XW], bf16)
                    nc.scalar.copy(xb[0:ctr, :], s(slice(0, ctr), slice(0, XW)))

                    pt = pspool.tile([128, W], f32)
                    for j in range(DYS):
                        nc.tensor.matmul(
                            pt[:],
                            wtile[0:ctr, j * 128 : (j + 1) * 128],
                            xb[0:ctr, j : j + W],
                            start=(j == 0),
                            stop=(j == DYS - 1),
                        )
                    nc.vector.tensor_copy(otb[0:nrows, t, :], pt[0:nrows, :])
                done_otb[b] = otb

                # Output DMAs for the PREVIOUS sample: all its evictions
                # finished long ago, so these descriptor-gens never block
                # the GpSimd DGE waiting on evict semaphores, keeping the
                # next samples' input DMAs flowing.
                if b - 1 in done_otb:
                    flush_outputs(b - 1)

            flush_outputs(BPC - 1)
    nc.compile()
    return nc


def _get_program():
    global _CACHED_NC
    if _CACHED_NC is None:
        _CACHED_NC = _build_program()
    return _CACHED_NC


def _run(grid_spikes, distance_weights, trace=False):
    grid_spikes = np.ascontiguousarray(np.asarray(grid_spikes, dtype=np.float32))
    distance_weights = np.asarray(distance_weights, dtype=np.float32)
    assert grid_spikes.shape == (B, H, W), grid_spikes.shape
    wb_np = _build_band_weights(distance_weights)

    nc = _get_program()
    in_maps = [
        {
            "x": np.ascontiguousarray(grid_spikes[i * BPC : (i + 1) * BPC]),
            "wb": wb_np,
        }
        for i in range(NCORES)
    ]
    res = run_bass_kernel_spmd(nc, in_maps, list(range(NCORES)), trace=trace)
    out = np.concatenate([res.results[i]["y"] for i in range(NCORES)], axis=0)
    return out.astype(np.float32, copy=False), res


def kernel(grid_spikes, distance_weights):
    out, _ = _run(grid_spikes, distance_weights, trace=False)
    return out


def kernel_traced(grid_spikes, distance_weights):
    out, res = _run(grid_spikes, distance_weights, trace=True)
    return out, res


# revision 15
# speedup vs baseline: 1.8410x; 1.0267x over previous
"""Trainium2 Bass kernel for the LocalConnectivity diamond-ring stencil.

out[b, x, y] = sum_{1<=|dx|+|dy|<=5} w[|dx|+|dy|-1] * in[b, (x+dx)%512, (y+dy)%512]

Strategy
--------
Data-parallel over batch: 64 samples -> 8 cores x 8 samples. Per sample the
512x512 grid is processed in 5 row-tiles (~103 output rows each). The whole
60-tap stencil runs on the TensorEngine as 11 PSUM-accumulating matmuls, one
per horizontal shift dy in [-5, 5]:

  psum[p, f] += sum_c  WB_dy[c, p] * X[c, f + dy_idx]

where X is the input tile with 5 halo rows on each side (contraction dim =
nrows+10 partitions) and 5 circular halo columns on each side (horizontal
shifts become free-dim AP offsets), and WB_dy is the banded Toeplitz matrix
holding the vertical taps of kernel column dy: WB_dy[c, p] = K(c-p-5, dy).

v2: all-bf16 matmuls. f32r weights cannot use FWL or the background weight
buffer, which exposed ~157ns of LDWEIGHTS serialization per matmul (~68us
of the 200us baseline). bf16 weights padded to the full 128 columns satisfy
the FWL trigger (NumWeights==128, non-fp32) and LDWEIGHTS prefetches into
the background weight buffer during the previous matmul. Mixed bf16 x f32r
is rejected by the NEFF compiler, so the moving data is converted f32->bf16
on ScalarE (one extra 113x522 copy per tile, ~0.5us, off the critical
engine). PSUM rows nrows..127 accumulate garbage from the zero-padded band
columns and are never evicted. A dozen warmup matmuls on the weight tile
ramp the PE p-state while the first input tiles are still in flight on DMA.

Bulk HBM traffic is issued from GpSimd (software DGE - the only DGE that
fans transfers out across all 16 SDMA engines; the sync/scalar HW-DGE queues
each serialize on one SDMA engine at ~15-18 GB/s). Transfers stay per-tile
so consecutive DMAs round-robin onto different SDMA queues. Circular column
halos are filled by on-chip ScalarE copies; PSUM eviction runs on VectorE.
"""

import numpy as np
import ml_dtypes

import concourse.bass as bass
import concourse.bacc as bacc
import concourse.mybir as mybir
from concourse import tile
from concourse.bass_utils import run_bass_kernel_spmd

B, H, W = 64, 512, 512
NCORES = 8
BPC = B // NCORES  # samples per core
MAXD = 5
HALO = MAXD
DYS = 2 * MAXD + 1  # 11 horizontal shifts
TR = 103  # rows per tile (last tile: 100)
ROW_TILES = [(0, 103), (103, 103), (206, 103), (309, 103), (412, 100)]
XW = W + 2 * HALO  # 522
WARMUP_MM = 12


def _build_band_weights(dw: np.ndarray) -> np.ndarray:
    """[128, 11*128] bf16: WB[c, j*128 + p] = K(c-p-5, j-5)."""
    wb = np.zeros((128, DYS, 128), dtype=np.float32)
    p = np.arange(128)
    for j in range(DYS):
        dy = j - MAXD
        for dx in range(-MAXD, MAXD + 1):
            d = abs(dx) + abs(dy)
            if 1 <= d <= MAXD:
                c = p + dx + HALO
                valid = (c >= 0) & (c < 128)
                wb[c[valid], j, p[valid]] = dw[d - 1]
    return np.ascontiguousarray(
        wb.reshape(128, DYS * 128).astype(ml_dtypes.bfloat16)
    )


_CACHED_NC = None


def _build_program():
    f32 = mybir.dt.float32
    bf16 = mybir.dt.bfloat16

    nc = bacc.Bacc(None, target_bir_lowering=False)
    x = nc.dram_tensor("x", [BPC, H, W], bf16, kind="ExternalInput")
    wb = nc.dram_tensor("wb", [128, DYS * 128], bf16, kind="ExternalInput")
    y = nc.dram_tensor("y", [BPC, H, W], f32, kind="ExternalOutput")

    with tile.TileContext(nc) as tc:
        with (
            tc.tile_pool(name="wpool", bufs=1) as wpool,
            tc.tile_pool(name="xmpool", bufs=5) as xmpool,
            tc.tile_pool(name="xepool", bufs=6) as xepool,
            tc.tile_pool(name="opool", bufs=4) as opool,
            tc.tile_pool(name="pspool", bufs=7, space=bass.MemorySpace.PSUM) as pspool,
        ):
            wtile = wpool.tile([128, DYS * 128], bf16)
            nc.gpsimd.dma_start(wtile[:], wb[:])

            # PE p-state warmup while the first input tiles are on DMA.
            wpt = pspool.tile([128, W], f32, tag="warm", bufs=1)
            for _ in range(WARMUP_MM):
                nc.tensor.matmul(
                    wpt[:],
                    wtile[0:128, 0:128],
                    wtile[0:128, 0:W],
                    start=True,
                    stop=True,
                )

            def issue_inputs(b):
                """Input DMAs for sample b. 2-sample lookahead keeps these
                ahead of the output DMAs on the in-order GpSimd DGE, so the
                next samples' transfers overlap this sample's compute."""
                xt0 = xepool.tile([128, XW], bf16, tag="xt0", name="xt0")
                #   rows 507..511 then 0..107
                nc.sync.dma_start(
                    xt0[0:HALO, HALO : HALO + W], x[b, H - HALO : H, :]
                )
                nc.gpsimd.dma_start(
                    xt0[HALO : HALO + 108, HALO : HALO + W], x[b, 0:108, :]
                )
                # interior tiles t=1..3: per-tile DMAs (merging them onto
                # one logical queue was measured 1.8x slower end-to-end)
                xtm = xmpool.tile([128, 3, XW], bf16, name="xtm")
                for tt in range(3):
                    r0 = TR * (tt + 1)
                    nc.gpsimd.dma_start(
                        xtm[0:113, tt, HALO : HALO + W],
                        x[b, r0 - HALO : r0 + 108, :],
                    )
                xt4 = xepool.tile([128, XW], bf16, tag="xt4", name="xt4")
                #   rows 407..511 then 0..4
                nc.gpsimd.dma_start(
                    xt4[0:105, HALO : HALO + W], x[b, 4 * TR - HALO : H, :]
                )
                nc.sync.dma_start(xt4[105:110, HALO : HALO + W], x[b, 0:HALO, :])
                return xt0, xtm, xt4

            pending = [issue_inputs(0), issue_inputs(1)]
            done_otb = {}

            def flush_outputs(bb):
                ob = done_otb.pop(bb)
                for t, (r0, nrows) in enumerate(ROW_TILES):
                    nc.gpsimd.dma_start(
                        y[bb, r0 : r0 + nrows, :], ob[0:nrows, t, :]
                    )

            for b in range(BPC):
                xt0, xtm, xt4 = pending[b]
                if b + 2 < BPC:
                    pending.append(issue_inputs(b + 2))

                # ---- per tile: halo fill + f32->bf16 convert (ScalarE,
                # issued per-tile so tile t's matmuls depend only on tile t's
                # own DMA), 11 accumulating matmuls, eviction, output DMA ----
                otb = opool.tile([128, 5, W], f32)
                for t, (r0, nrows) in enumerate(ROW_TILES):
                    ctr = nrows + 2 * HALO

                    def s(rs, cs, _t=t, _xt0=xt0, _xt4=xt4, _xtm=xtm):
                        if _t == 0:
                            return _xt0[rs, cs]
                        if _t == 4:
                            return _xt4[rs, cs]
                        return _xtm[rs, _t - 1, cs]

                    nc.scalar.copy(
                        s(slice(0, ctr), slice(0, HALO)),
                        s(slice(0, ctr), slice(W, W + HALO)),
                    )
                    nc.scalar.copy(
                        s(slice(0, ctr), slice(HALO + W, XW)),
                        s(slice(0, ctr), slice(HALO, 2 * HALO)),
                    )
                    pt = pspool.tile([128, W], f32)
                    for j in range(DYS):
                        nc.tensor.matmul(
                            pt[:],
                            wtile[0:ctr, j * 128 : (j + 1) * 128],
                            s(slice(0, ctr), slice(j, j + W)),
                            start=(j == 0),
                            stop=(j == DYS - 1),
                        )
                    nc.vector.tensor_copy(otb[0:nrows, t, :], pt[0:nrows, :])
                done_otb[b] = otb

                # Output DMAs for the PREVIOUS sample: all its evictions
                # finished long ago, so these descriptor-gens never block
                # the GpSimd DGE waiting on evict semaphores, keeping the
                # next samples' input DMAs flowing.
                if b - 1 in done_otb:
                    flush_outputs(b - 1)

            flush_outputs(BPC - 1)
    nc.compile()
    return nc


def _get_program():
    global _CACHED_NC
    if _CACHED_NC is None:
        _CACHED_NC = _build_program()
    return _CACHED_NC


def _run(grid_spikes, distance_weights, trace=False):
    grid_spikes = np.asarray(grid_spikes)
    distance_weights = np.asarray(distance_weights, dtype=np.float32)
    assert grid_spikes.shape == (B, H, W), grid_spikes.shape
    wb_np = _build_band_weights(distance_weights)
    # host-side f32 -> bf16: halves input HBM traffic and removes the
    # on-chip conversion from the per-tile critical chain
    x16 = grid_spikes.astype(ml_dtypes.bfloat16)

    nc = _get_program()
    in_maps = [
        {
            "x": np.ascontiguousarray(x16[i * BPC : (i + 1) * BPC]),
            "wb": wb_np,
        }
        for i in range(NCORES)
    ]
    res = run_bass_kernel_spmd(nc, in_maps, list(range(NCORES)), trace=trace)
    out = np.concatenate([res.results[i]["y"] for i in range(NCORES)], axis=0)
    return out.astype(np.float32, copy=False), res


def kernel(grid_spikes, distance_weights):
    out, _ = _run(grid_spikes, distance_weights, trace=False)
    return out


def kernel_traced(grid_spikes, distance_weights):
    out, res = _run(grid_spikes, distance_weights, trace=True)
    return out, res


# revision 16
# speedup vs baseline: 2.0073x; 1.0903x over previous
"""Trainium2 Bass kernel for the LocalConnectivity diamond-ring stencil.

out[b, x, y] = sum_{1<=|dx|+|dy|<=5} w[|dx|+|dy|-1] * in[b, (x+dx)%512, (y+dy)%512]

Strategy
--------
Data-parallel over batch: 64 samples -> 8 cores x 8 samples. Per sample the
512x512 grid is processed in 5 row-tiles (~103 output rows each). The whole
60-tap stencil runs on the TensorEngine as 11 PSUM-accumulating matmuls, one
per horizontal shift dy in [-5, 5]:

  psum[p, f] += sum_c  WB_dy[c, p] * X[c, f + dy_idx]

where X is the input tile with 5 halo rows on each side (contraction dim =
nrows+10 partitions) and 5 circular halo columns on each side (horizontal
shifts become free-dim AP offsets), and WB_dy is the banded Toeplitz matrix
holding the vertical taps of kernel column dy: WB_dy[c, p] = K(c-p-5, dy).

v2: all-bf16 matmuls. f32r weights cannot use FWL or the background weight
buffer, which exposed ~157ns of LDWEIGHTS serialization per matmul (~68us
of the 200us baseline). bf16 weights padded to the full 128 columns satisfy
the FWL trigger (NumWeights==128, non-fp32) and LDWEIGHTS prefetches into
the background weight buffer during the previous matmul. Mixed bf16 x f32r
is rejected by the NEFF compiler, so the moving data is converted f32->bf16
on ScalarE (one extra 113x522 copy per tile, ~0.5us, off the critical
engine). PSUM rows nrows..127 accumulate garbage from the zero-padded band
columns and are never evicted. A dozen warmup matmuls on the weight tile
ramp the PE p-state while the first input tiles are still in flight on DMA.

Bulk HBM traffic is issued from GpSimd (software DGE - the only DGE that
fans transfers out across all 16 SDMA engines; the sync/scalar HW-DGE queues
each serialize on one SDMA engine at ~15-18 GB/s). Transfers stay per-tile
so consecutive DMAs round-robin onto different SDMA queues. Circular column
halos are filled by on-chip ScalarE copies; PSUM eviction runs on VectorE.
"""

import numpy as np
import ml_dtypes

import concourse.bass as bass
import concourse.bacc as bacc
import concourse.mybir as mybir
from concourse import tile
from concourse.bass_utils import run_bass_kernel_spmd

B, H, W = 64, 512, 512
NCORES = 8
BPC = B // NCORES  # samples per core
MAXD = 5
HALO = MAXD
DYS = 2 * MAXD + 1  # 11 horizontal shifts
TR = 103  # rows per tile (last tile: 100)
ROW_TILES = [(0, 103), (103, 103), (206, 103), (309, 103), (412, 100)]
XW = W + 2 * HALO  # 522
WARMUP_MM = 12


def _build_band_weights(dw: np.ndarray) -> np.ndarray:
    """[128, 11*128] bf16: WB[c, j*128 + p] = K(c-p-5, j-5)."""
    wb = np.zeros((128, DYS, 128), dtype=np.float32)
    p = np.arange(128)
    for j in range(DYS):
        dy = j - MAXD
        for dx in range(-MAXD, MAXD + 1):
            d = abs(dx) + abs(dy)
            if 1 <= d <= MAXD:
                c = p + dx + HALO
                valid = (c >= 0) & (c < 128)
                wb[c[valid], j, p[valid]] = dw[d - 1]
    return np.ascontiguousarray(
        wb.reshape(128, DYS * 128).astype(ml_dtypes.bfloat16)
    )


_CACHED_NC = None


def _build_program():
    f32 = mybir.dt.float32
    bf16 = mybir.dt.bfloat16

    nc = bacc.Bacc(None, target_bir_lowering=False)
    x = nc.dram_tensor("x", [BPC, H, W], bf16, kind="ExternalInput")
    wb = nc.dram_tensor("wb", [128, DYS * 128], bf16, kind="ExternalInput")
    y = nc.dram_tensor("y", [BPC, H, W], f32, kind="ExternalOutput")

    with tile.TileContext(nc) as tc:
        with (
            tc.tile_pool(name="wpool", bufs=1) as wpool,
            tc.tile_pool(name="xmpool", bufs=5) as xmpool,
            tc.tile_pool(name="xepool", bufs=6) as xepool,
            tc.tile_pool(name="opool", bufs=4) as opool,
            tc.tile_pool(name="pspool", bufs=7, space=bass.MemorySpace.PSUM) as pspool,
        ):
            wtile = wpool.tile([128, DYS * 128], bf16)
            nc.gpsimd.dma_start(wtile[:], wb[:])

            # PE p-state warmup while the first input tiles are on DMA.
            wpt = pspool.tile([128, W], f32, tag="warm", bufs=1)
            for _ in range(WARMUP_MM):
                nc.tensor.matmul(
                    wpt[:],
                    wtile[0:128, 0:128],
                    wtile[0:128, 0:W],
                    start=True,
                    stop=True,
                )

            def issue_inputs(b):
                """Input DMAs for sample b. 2-sample lookahead keeps these
                ahead of the output DMAs on the in-order GpSimd DGE, so the
                next samples' transfers overlap this sample's compute."""
                xt0 = xepool.tile([128, XW], bf16, tag="xt0", name="xt0")
                #   rows 507..511 then 0..107
                nc.sync.dma_start(
                    xt0[0:HALO, HALO : HALO + W], x[b, H - HALO : H, :]
                )
                nc.gpsimd.dma_start(
                    xt0[HALO : HALO + 108, HALO : HALO + W], x[b, 0:108, :]
                )
                # interior tiles t=1..3: per-tile DMAs (merging them onto
                # one logical queue was measured 1.8x slower end-to-end)
                xtm = xmpool.tile([128, 3, XW], bf16, name="xtm")
                for tt in range(3):
                    r0 = TR * (tt + 1)
                    nc.gpsimd.dma_start(
                        xtm[0:113, tt, HALO : HALO + W],
                        x[b, r0 - HALO : r0 + 108, :],
                    )
                xt4 = xepool.tile([128, XW], bf16, tag="xt4", name="xt4")
                #   rows 407..511 then 0..4
                nc.gpsimd.dma_start(
                    xt4[0:105, HALO : HALO + W], x[b, 4 * TR - HALO : H, :]
                )
                nc.sync.dma_start(xt4[105:110, HALO : HALO + W], x[b, 0:HALO, :])
                return xt0, xtm, xt4

            pending = [issue_inputs(0), issue_inputs(1)]
            done_otb = {}

            def issue_output(bb, t):
                r0, nrows = ROW_TILES[t]
                nc.gpsimd.dma_start(
                    y[bb, r0 : r0 + nrows, :], done_otb[bb][0:nrows, t, :]
                )

            for b in range(BPC):
                xt0, xtm, xt4 = pending[b]
                if b + 2 < BPC:
                    pending.append(issue_inputs(b + 2))

                # ---- per tile: halo fill + f32->bf16 convert (ScalarE,
                # issued per-tile so tile t's matmuls depend only on tile t's
                # own DMA), 11 accumulating matmuls, eviction, output DMA ----
                otb = opool.tile([128, 5, W], f32)
                for t, (r0, nrows) in enumerate(ROW_TILES):
                    ctr = nrows + 2 * HALO

                    def s(rs, cs, _t=t, _xt0=xt0, _xt4=xt4, _xtm=xtm):
                        if _t == 0:
                            return _xt0[rs, cs]
                        if _t == 4:
                            return _xt4[rs, cs]
                        return _xtm[rs, _t - 1, cs]

                    nc.scalar.copy(
                        s(slice(0, ctr), slice(0, HALO)),
                        s(slice(0, ctr), slice(W, W + HALO)),
                    )
                    nc.scalar.copy(
                        s(slice(0, ctr), slice(HALO + W, XW)),
                        s(slice(0, ctr), slice(HALO, 2 * HALO)),
                    )
                    pt = pspool.tile([128, W], f32)
                    for j in range(DYS):
                        nc.tensor.matmul(
                            pt[:],
                            wtile[0:ctr, j * 128 : (j + 1) * 128],
                            s(slice(0, ctr), slice(j, j + W)),
                            start=(j == 0),
                            stop=(j == DYS - 1),
                        )
                    nc.vector.tensor_copy(otb[0:nrows, t, :], pt[0:nrows, :])
                    # output DMA for the PREVIOUS sample's same tile: its
                    # eviction is long done, so the descriptor-gen never
                    # blocks the GpSimd DGE on an evict semaphore, and the
                    # transfers stream during compute instead of piling up
                    if b >= 1:
                        issue_output(b - 1, t)
                done_otb[b] = otb

            for t in range(len(ROW_TILES)):
                issue_output(BPC - 1, t)
    nc.compile()
    return nc


def _get_program():
    global _CACHED_NC
    if _CACHED_NC is None:
        _CACHED_NC = _build_program()
    return _CACHED_NC


def _run(grid_spikes, distance_weights, trace=False):
    grid_spikes = np.asarray(grid_spikes)
    distance_weights = np.asarray(distance_weights, dtype=np.float32)
    assert grid_spikes.shape == (B, H, W), grid_spikes.shape
    wb_np = _build_band_weights(distance_weights)
    # host-side f32 -> bf16: halves input HBM traffic and removes the
    # on-chip conversion from the per-tile critical chain
    x16 = grid_spikes.astype(ml_dtypes.bfloat16)

    nc = _get_program()
    in_maps = [
        {
            "x": np.ascontiguousarray(x16[i * BPC : (i + 1) * BPC]),
            "wb": wb_np,
        }
        for i in range(NCORES)
    ]
    res = run_bass_kernel_spmd(nc, in_maps, list(range(NCORES)), trace=trace)
    out = np.concatenate([res.results[i]["y"] for i in range(NCORES)], axis=0)
    return out.astype(np.float32, copy=False), res


def kernel(grid_spikes, distance_weights):
    out, _ = _run(grid_spikes, distance_weights, trace=False)
    return out


def kernel_traced(grid_spikes, distance_weights):
    out, res = _run(grid_spikes, distance_weights, trace=True)
    return out, res
